# revision 1
# baseline (speedup 1.0000x reference)
"""Trainium2 Bass kernel for nn_DTSHLoss_48189533061230.

Reference computation (B=384, BITS=128, NCLS=80):
    ip = u @ u.T
    s  = (yf @ yf.T) > 0            # similarity mask from binary labels
    triple[r,i,j] = clip(ip[r,i] - ip[r,j] - 0.5, -100, 50)
    sp = softplus(-triple)
    w  = pos[:, :, None] * neg[:, None, :]
    row_loss[r] = sum_ij(sp * w) / pair_count[r]        (rows with pos&neg)
    loss1 = mean over valid rows;  loss2 = 0.1 * mean((u - sign(u))**2)
    out = loss1 + loss2   (f32 scalar)

Structure exploited (ragged_sequence): with NCLS=80 random bits per row,
P(two rows share no class) ~ (3/4)^80 ~ 1e-10, so rows essentially never
have a negative partner => pair_count == 0 for all rows => loss1 == 0
exactly (w == 0 identically, count == 0). The host computes the
pos/neg masks (integer bookkeeping) to decide the ragged work schedule;
only rows with pair_count > 0 get O(B^2) device work. The always-on
device work is loss2, sharded 8 ways over rows of u (data parallel),
with the scalar partials all-reduced on the host during unshard.
"""

from contextlib import ExitStack

import numpy as np

import concourse.bass as bass
import concourse.bacc as bacc
import concourse.mybir as mybir
import concourse.tile as tile
from concourse.bass_utils import run_bass_kernel_spmd

N_CORES = 8
B, BITS, NCLS = 384, 128, 80
F32 = mybir.dt.float32
AF = mybir.ActivationFunctionType


def build_loss2_program(rows_per_core: int):
    """Per-core: partial = sum over its row shard of x^2 - 2|x|.

    sum((u - sign(u))^2) == sum(x^2) - 2*sum(|x|) + N for u without exact
    zeros; the constant N (and a correction for exact zeros, which the
    identity miscounts by +1 each) is integer host math added at unshard.
    DVE produces per-partition -sum(|x|) (negated abs reduce) and
    sum(x^2) (mul then reduce); three accumulating 1-column matmuls
    against the preamble const-1.0 column reduce across partitions into
    one PSUM scalar (nabs twice + sq_sum once = sq - 2|x|). Input "ut"
    is the row shard pre-transposed to [BITS, R] so rows live on the
    128-partition axis.
    """
    R = rows_per_core
    nc = bass.Bass(
        "TRN2", target_bir_lowering=False, debug=False, num_devices=N_CORES
    )
    ut = nc.dram_tensor("ut", [BITS, R], F32, kind="ExternalInput")
    out = nc.dram_tensor("partial", [1, 1], F32, kind="ExternalOutput")
    # the preamble-initialized ones column doubles as the matmul rhs;
    # -2*sum|x| comes from accumulating the negated abs-sums twice
    const1 = nc.const_aps.tensor(1.0, (128, 1), F32)

    # Emitted directly into main (no nc.Block()): skips the block entry/exit
    # all-engine barriers (~0.8us on the measured critical path). Ordering is
    # purely semaphore-based.
    with (
        nc.sbuf_tensor([BITS, R], F32) as x,
        nc.sbuf_tensor([BITS, R], F32) as sq,
        nc.sbuf_tensor([BITS, 1], F32) as sq_sum,
        nc.sbuf_tensor([BITS, 1], F32) as nabs,
        nc.sbuf_tensor([1, 1], F32) as res,
        nc.psum_tensor([1, 1], F32) as ps,
        nc.semaphore() as dma_sem,
        nc.semaphore() as dve_sem,
        nc.semaphore() as mm_sem,
        nc.semaphore() as res_sem,
    ):
        nc.sync.dma_start(x[:], ut[:]).then_inc(dma_sem, 16)

        nc.vector.wait_ge(dma_sem, 16)
        # mul first: its completion signal lands while the abs-reduce runs,
        # so the same-engine RAW self-wait before the sq reduce is free
        nc.vector.tensor_mul(sq[:], x[:], x[:]).then_inc(dve_sem, 1)
        nc.vector.reduce_sum(
            nabs[:], x[:], axis=mybir.AxisListType.X,
            apply_absolute_value=True, negate=True,
        ).then_inc(dve_sem, 1)
        nc.vector.wait_ge(dve_sem, 1)
        nc.vector.reduce_sum(
            sq_sum[:], sq[:], axis=mybir.AxisListType.X
        ).then_inc(dve_sem, 1)

        # nabs pair starts as soon as the abs-reduce lands (overlapping the
        # sq reduce); the sq matmul joins the same PSUM accumulation group
        nc.tensor.wait_ge(dve_sem, 2)
        nc.tensor.matmul(ps[:], nabs[:], const1, start=True, stop=False)
        nc.tensor.matmul(ps[:], nabs[:], const1, start=False, stop=False)
        nc.tensor.wait_ge(dve_sem, 3)
        nc.tensor.matmul(
            ps[:], sq_sum[:], const1, start=False, stop=True
        ).then_inc(mm_sem, 1)

        nc.vector.wait_ge(mm_sem, 1)
        nc.vector.tensor_copy(res[:], ps[:]).then_inc(res_sem, 1)

        nc.sync.wait_ge(res_sem, 1)
        nc.sync.dma_start(out[:], res[:], single_packet=True).then_inc(dma_sem, 16)

    return nc


_program_cache: dict = {}


def _cached(key, builder, *args):
    if key not in _program_cache:
        _program_cache[key] = builder(*args)
    return _program_cache[key]


def kernel(u: np.ndarray, y: np.ndarray) -> np.ndarray:
    u = np.ascontiguousarray(np.asarray(u, dtype=np.float32))
    y = np.asarray(y, dtype=np.int32)
    assert u.shape == (B, BITS) and y.shape == (B, NCLS)

    # ---- host-side ragged schedule bookkeeping (integer label math) ----
    yy = y.astype(np.int64) @ y.astype(np.int64).T
    pos = yy > 0  # [B, B] bool, includes self unless the row is all-zero
    n_pos = pos.sum(1)
    n_neg = B - n_pos
    pair_count = (n_pos * n_neg).astype(np.float64)
    valid = pair_count > 0
    n_valid = int(valid.sum())

    # ---- loss2: always on device, 8-way data parallel over rows ----
    R = B // N_CORES
    uT = np.ascontiguousarray(u.T)  # [BITS, B]
    in_maps = [
        {"ut": np.ascontiguousarray(uT[:, c * R : (c + 1) * R])}
        for c in range(N_CORES)
    ]
    nc = _cached(("loss2", R), build_loss2_program, R)
    res = run_bass_kernel_spmd(nc, in_maps, core_ids=list(range(N_CORES)))
    partials = [float(r["partial"][0, 0]) for r in res.results]
    n_zero = int(np.count_nonzero(u == 0.0))  # x^2-2|x|+1 miscounts these as 1
    total = float(np.sum(partials)) + float(B * BITS - n_zero)
    loss2 = np.float32(0.1 * (total / (B * BITS)))

    loss1 = np.float32(0.0)
    if n_valid > 0:
        loss1 = _loss1_device(u, pos, pair_count, valid, n_valid)

    return np.array(loss1 + loss2, dtype=np.float32)


def build_loss1_program(nr: int):
    """Per-core loss1 partial over `nr` assigned anchor rows (padded).

    For each anchor q with ip-row a (a_i = <u_r, u_i>):
        U[i, j]  = a_j - a_i + 0.5                 == -(triple[r, i, j])
        C        = clip(U, -50, 100)               (mirror of clip(t,-100,50))
        SP       = softplus(C)                     == softplus(-clip(t))
        contrib  = sum_ij posw_i * SP[i, j] * neg_j   (posw = pos/pair_count)
    partial = sum over anchors; the host all-reduces partials / count.

    Inputs: "uT" [BITS, B] full u^T (replicated), "anch" [BITS, nr] anchor
    columns, "poswc" [128, 3, nr] pos/pc masks partition-chunked, "negwT"
    [nr, B] neg mask rows. Pad anchors get zero masks.
    """
    nc = bacc.Bacc(
        "TRN2", target_bir_lowering=False, debug=False, num_devices=N_CORES
    )
    uT = nc.dram_tensor("uT", [BITS, B], F32, kind="ExternalInput")
    anch = nc.dram_tensor("anch", [BITS, nr], F32, kind="ExternalInput")
    poswc = nc.dram_tensor("poswc", [128, 3, nr], F32, kind="ExternalInput")
    negwT = nc.dram_tensor("negwT", [nr, B], F32, kind="ExternalInput")
    out = nc.dram_tensor("l1partial", [1, 1], F32, kind="ExternalOutput")
    NCH = B // 128  # 3 partition chunks of the i axis

    with tile.TileContext(nc) as tc, ExitStack() as ctx:
        io = ctx.enter_context(tc.tile_pool(name="io", bufs=1))
        stat = ctx.enter_context(tc.tile_pool(name="stat", bufs=1))
        work = ctx.enter_context(tc.tile_pool(name="work", bufs=4))
        psb = ctx.enter_context(tc.tile_pool(name="psb", bufs=2, space="PSUM"))

        sb_uT = io.tile([BITS, B], F32)
        nc.sync.dma_start(sb_uT[:], uT[:])
        sb_anch = io.tile([BITS, nr], F32)
        nc.sync.dma_start(sb_anch[:], anch[:])
        sb_posw = io.tile([128, NCH, nr], F32)
        nc.sync.dma_start(sb_posw[:], poswc[:])

        ones_col = stat.tile([128, 1], F32)
        nc.vector.memset(ones_col[:], 1.0)

        # aPm[c][:, q] = ip chunk values minus 0.5 (per-partition bias for U)
        aPm = []
        for c in range(NCH):
            ps_ip = psb.tile([128, nr], F32, tag="ps_ip")
            nc.tensor.matmul(
                ps_ip[:], sb_uT[:, c * 128 : (c + 1) * 128], sb_anch[:],
                start=True, stop=True,
            )
            a = stat.tile([128, nr], F32, tag=f"aPm{c}")
            nc.vector.tensor_scalar_sub(a[:], ps_ip[:], 0.5)
            aPm.append(a)

        # aT[q, :] = full ip row per anchor; staged to DRAM because only DRAM
        # APs support the stride-0 partition-broadcast reads used below
        ps_aT = psb.tile([nr, B], F32, tag="ps_aT")
        nc.tensor.matmul(ps_aT[:], sb_anch[:], sb_uT[:], start=True, stop=True)
        sb_aT = stat.tile([nr, B], F32)
        nc.vector.tensor_copy(sb_aT[:], ps_aT[:])
        aT_dram = nc.dram_tensor("aT_scratch", [nr, B], F32)
        nc.sync.dma_start(aT_dram.ap(), sb_aT[:])

        # per-(anchor, chunk) partial column sums land here
        v_all = stat.tile([128, NCH * nr], F32)

        def row_bcast_ap(ap_row):
            # [1, B] row -> [128, B] partition-broadcast source AP for DMA
            return bass.AP(
                tensor=ap_row.tensor,
                offset=ap_row.offset,
                ap=[[0, 128]] + list(ap_row.ap)[1:],
            )

        for q in range(nr):
            # broadcast a (row q) and neg (row q) across all 128 partitions
            # via stride-0 DMA reads; Tile prefetches these ahead of compute
            xa = work.tile([128, B], F32, tag="xa")
            nc.sync.dma_start(xa[:], row_bcast_ap(aT_dram.ap()[q : q + 1, :]))
            xn = work.tile([128, B], F32, tag="xn")
            nc.sync.dma_start(xn[:], row_bcast_ap(negwT.ap()[q : q + 1, :]))
            for c in range(NCH):
                u_t = work.tile([128, B], F32, tag="u_t")
                # U = a_j - (a_i - 0.5)
                nc.vector.tensor_scalar(
                    u_t[:], xa[:], aPm[c][:, q : q + 1], None,
                    mybir.AluOpType.subtract,
                )
                cl = work.tile([128, B], F32, tag="cl")
                nc.vector.tensor_scalar(
                    cl[:], u_t[:], 100.0, -50.0,
                    mybir.AluOpType.min, mybir.AluOpType.max,
                )
                # softplus(cl) = max(cl,0) + ln(1 + exp(-|cl|)); the Softplus
                # ACT table slot is unnamed in this toolchain, so decompose
                # (Abs/Exp/Ln all live in the natural_log_exp_and_others table)
                ab = work.tile([128, B], F32, tag="ab")
                nc.scalar.activation(ab[:], cl[:], AF.Abs)
                ex = work.tile([128, B], F32, tag="ex")
                nc.scalar.activation(ex[:], ab[:], AF.Exp, scale=-1.0)
                ln = work.tile([128, B], F32, tag="ln")
                nc.scalar.activation(ln[:], ex[:], AF.Ln, bias=1.0)
                rl = work.tile([128, B], F32, tag="rl")
                nc.vector.tensor_scalar_max(rl[:], cl[:], 0.0)
                sp = work.tile([128, B], F32, tag="sp")
                nc.vector.tensor_add(sp[:], ln[:], rl[:])
                w = work.tile([128, B], F32, tag="w")
                # w = posw_i * SP * neg_j
                nc.vector.scalar_tensor_tensor(
                    w[:], sp[:], sb_posw[:, c, q : q + 1], xn[:],
                    mybir.AluOpType.mult, mybir.AluOpType.mult,
                )
                nc.vector.reduce_sum(
                    v_all[:, q * NCH + c : q * NCH + c + 1], w[:],
                    axis=mybir.AxisListType.X,
                )

        vtot = stat.tile([128, 1], F32)
        nc.vector.reduce_sum(vtot[:], v_all[:], axis=mybir.AxisListType.X)
        ps_out = psb.tile([1, 1], F32, tag="ps_out")
        nc.tensor.matmul(ps_out[:], vtot[:], ones_col[:], start=True, stop=True)
        res = stat.tile([1, 1], F32)
        nc.vector.tensor_copy(res[:], ps_out[:])
        nc.sync.dma_start(out[:], res[:])

    nc.compile()
    return nc


def _loss1_device(u, pos, pair_count, valid, n_valid):
    """Shard valid anchor rows over the cores; run the loss1 program."""
    valid_rows = np.nonzero(valid)[0]
    nr = max(1, (n_valid + N_CORES - 1) // N_CORES)
    uT = np.ascontiguousarray(u.T)  # [BITS, B]

    posw_full = pos.astype(np.float64) / np.where(valid, pair_count, 1.0)[:, None]
    negw_full = 1.0 - pos.astype(np.float64)

    in_maps = []
    for c in range(N_CORES):
        rows = valid_rows[c * nr : (c + 1) * nr]
        anch = np.zeros((BITS, nr), np.float32)
        poswc = np.zeros((128, B // 128, nr), np.float32)
        negwT = np.zeros((nr, B), np.float32)
        for q, r in enumerate(rows):
            anch[:, q] = u[r]
            poswc[:, :, q] = posw_full[r].astype(np.float32).reshape(B // 128, 128).T
            negwT[q, :] = negw_full[r].astype(np.float32)
        in_maps.append(
            {
                "uT": uT,
                "anch": anch,
                "poswc": np.ascontiguousarray(poswc),
                "negwT": negwT,
            }
        )

    nc = _cached(("loss1", nr), build_loss1_program, nr)
    res = run_bass_kernel_spmd(nc, in_maps, core_ids=list(range(N_CORES)))
    partials = [float(r["l1partial"][0, 0]) for r in res.results]
    return np.float32(float(np.sum(partials)) / float(n_valid))



# revision 2
# speedup vs baseline: 1.4434x; 1.4434x over previous
"""Trainium2 Bass kernel for nn_DTSHLoss_48189533061230.

Reference computation (B=384, BITS=128, NCLS=80):
    ip = u @ u.T
    s  = (yf @ yf.T) > 0            # similarity mask from binary labels
    triple[r,i,j] = clip(ip[r,i] - ip[r,j] - 0.5, -100, 50)
    sp = softplus(-triple); w = pos[:,:,None] * neg[:,None,:]
    loss1 = mean over rows with pos&neg pairs;  loss2 = 0.1*mean((u-sign(u))^2)

Structure exploited (ragged_sequence): with NCLS=80 random bits per row,
P(two rows share no class) ~ (3/4)^80 ~ 1e-10, so pair_count == 0 for all
rows => loss1 == 0 exactly. The host does the integer label bookkeeping to
build the ragged schedule; only rows with pairs would get O(B^2) device
work (build_loss1_program, kept for completeness). The always-on device
work is loss2, sharded 8 ways over rows of u (data parallel), with the
scalar partials reduced on the host during unshard.

loss2 device program (per core, shard reshaped to x [128, 48] + canary col):
    DVE  stt1: accum -> t1[p] = sum_c x^2           (fused multiply+row-accum)
    DVE  stt2: accum -> a1[p] = sum_c max(-x, x)    (= sum |x|)
    Pool copy: canary column -> colcat[:, 2]
    Sync DMA out colcat [128, 3]
Host: partial = sum(t1) - 2*sum(a1);  loss2 = 0.1*(sum partials + N - n0)/N.

The out-DMA completion is NOT awaited on-device (awaiting the 16 per-engine
HBM write receipts costs ~5us). Instead a per-attempt random canary vector
rides the full input-DMA -> SBUF -> output-DMA pipeline; the host validates
it bit-exactly and reruns the program on mismatch (each output row's 12
bytes travel in one DMA packet, so a row's canary validates its data).

The Bass-constructor preamble (const memsets, register inits, all-engine
barrier) is pruned from the emitted program: this kernel uses no const APs
or register ops, and the NRT preamble already synchronizes the engines.
"""

import numpy as np

import concourse.bass as bass
import concourse.mybir as mybir
from concourse.bass_utils import run_bass_kernel_spmd

N_CORES = 8
B, BITS, NCLS = 384, 128, 80
F32 = mybir.dt.float32
ALU = mybir.AluOpType
AF = mybir.ActivationFunctionType

P = 128               # SBUF partitions used by the loss2 program
RPC = B // N_CORES    # 48 anchor rows per core
RF = BITS * RPC // P  # 48 free-dim columns per core


def build_loss2_program(rows_per_core: int = RPC):
    R = BITS * rows_per_core // P
    nc = bass.Bass(
        "TRN2", target_bir_lowering=False, debug=False, num_devices=N_CORES
    )

    # Drop the constructor-emitted preamble (const memsets + register inits +
    # all-engine barrier): nothing in this program reads const APs or engine
    # registers, and NRT's own preamble barrier already gates user code.
    insts = nc.main_func.blocks[0].instructions
    insts[:] = [
        ins
        for ins in insts
        if type(ins).__name__
        not in ("InstMemset", "InstDrain", "InstEventSemaphore", "InstRegisterMove")
    ]

    ut = nc.dram_tensor("ut", [P, R + 1], F32, kind="ExternalInput")
    out = nc.dram_tensor("partial", [P, 3], F32, kind="ExternalOutput")

    with (
        nc.sbuf_tensor([P, R + 1], F32) as x,
        nc.sbuf_tensor([P, R], F32) as junk,
        nc.sbuf_tensor([P, R], F32) as junk2,
        nc.sbuf_tensor([P, 3], F32) as colcat,
        nc.semaphore() as dma_sem,
        nc.semaphore() as v_sem,
    ):
        nc.sync.dma_start(x[:], ut[:]).then_inc(dma_sem, 16)

        nc.vector.wait_ge(dma_sem, 16)
        nc.vector.scalar_tensor_tensor(
            junk[:], x[:, 0:R], 1.0, x[:, 0:R], ALU.mult, ALU.mult,
            accum_out=colcat[:, 0:1],
        ).then_inc(v_sem, 1)
        nc.vector.scalar_tensor_tensor(
            junk2[:], x[:, 0:R], -1.0, x[:, 0:R], ALU.mult, ALU.max,
            accum_out=colcat[:, 1:2],
        ).then_inc(v_sem, 1)

        nc.gpsimd.wait_ge(dma_sem, 16)
        nc.gpsimd.tensor_copy(colcat[:, 2:3], x[:, R : R + 1]).then_inc(v_sem, 1)

        nc.sync.wait_ge(v_sem, 3)
        nc.sync.dma_start(out[:], colcat[:]).then_inc(dma_sem, 16)

    return nc


_program_cache: dict = {}


def _cached(key, builder, *args):
    if key not in _program_cache:
        _program_cache[key] = builder(*args)
    return _program_cache[key]


def _loss2_device(u: np.ndarray) -> float:
    """Run the sharded loss2 program; returns sum((x-sign(x))^2) over all of u."""
    rng = np.random.default_rng()
    nc = _cached(("loss2", RPC), build_loss2_program, RPC)
    shards = [
        u[c * RPC : (c + 1) * RPC, :].reshape(P, RF) for c in range(N_CORES)
    ]

    total = None
    for _attempt in range(8):
        cans = [
            rng.standard_normal((P, 1)).astype(np.float32) for _ in range(N_CORES)
        ]
        in_maps = [
            {"ut": np.ascontiguousarray(np.concatenate([s, c], axis=1))}
            for s, c in zip(shards, cans)
        ]
        res = run_bass_kernel_spmd(nc, in_maps, core_ids=list(range(N_CORES)))
        ok = True
        tot = 0.0
        for c in range(N_CORES):
            arr = np.asarray(res.results[c]["partial"], dtype=np.float64)
            if not np.array_equal(
                np.asarray(res.results[c]["partial"])[:, 2:3], cans[c]
            ):
                ok = False  # stale/partial output observed; rerun
                break
            tot += arr[:, 0].sum() - 2.0 * arr[:, 1].sum()
        if ok:
            total = tot
            break
    assert total is not None, "loss2 device outputs failed canary validation"

    n_zero = int(np.count_nonzero(u == 0.0))  # x^2-2|x|+1 miscounts these as 1
    return float(total) + float(B * BITS - n_zero)


def kernel(u: np.ndarray, y: np.ndarray) -> np.ndarray:
    u = np.ascontiguousarray(np.asarray(u, dtype=np.float32))
    y = np.asarray(y, dtype=np.int32)
    assert u.shape == (B, BITS) and y.shape == (B, NCLS)

    # ---- host-side ragged schedule bookkeeping (integer label math) ----
    yy = y.astype(np.int64) @ y.astype(np.int64).T
    pos = yy > 0  # [B, B] bool, includes self unless the row is all-zero
    n_pos = pos.sum(1)
    n_neg = B - n_pos
    pair_count = (n_pos * n_neg).astype(np.float64)
    valid = pair_count > 0
    n_valid = int(valid.sum())

    # ---- loss2: always on device, 8-way data parallel over rows ----
    total = _loss2_device(u)
    loss2 = np.float32(0.1 * (total / (B * BITS)))

    loss1 = np.float32(0.0)
    if n_valid > 0:
        loss1 = _loss1_device(u, pos, pair_count, valid, n_valid)

    return np.array(loss1 + loss2, dtype=np.float32)


# ---------------------------------------------------------------------------
# loss1 path: only reachable if some row has both a positive and a negative
# partner (probability ~1e-10 per row pair set under the reference input
# distribution). Kept for correctness on adversarial inputs.
# ---------------------------------------------------------------------------


def build_loss1_program(nr: int):
    """Per-core loss1 partial over `nr` assigned anchor rows (padded).

    For each anchor q with ip-row a (a_i = <u_r, u_i>):
        U[i, j]  = a_j - a_i + 0.5                 == -(triple[r, i, j])
        C        = clip(U, -50, 100)               (mirror of clip(t,-100,50))
        SP       = softplus(C)                     == softplus(-clip(t))
        contrib  = sum_ij posw_i * SP[i, j] * neg_j   (posw = pos/pair_count)
    partial = sum over anchors; the host reduces partials / count.
    """
    from contextlib import ExitStack

    import concourse.bacc as bacc
    import concourse.tile as tile

    nc = bacc.Bacc(
        "TRN2", target_bir_lowering=False, debug=False, num_devices=N_CORES
    )
    uT = nc.dram_tensor("uT", [BITS, B], F32, kind="ExternalInput")
    anch = nc.dram_tensor("anch", [BITS, nr], F32, kind="ExternalInput")
    poswc = nc.dram_tensor("poswc", [128, 3, nr], F32, kind="ExternalInput")
    negwT = nc.dram_tensor("negwT", [nr, B], F32, kind="ExternalInput")
    out = nc.dram_tensor("l1partial", [1, 1], F32, kind="ExternalOutput")
    NCH = B // 128  # 3 partition chunks of the i axis

    with tile.TileContext(nc) as tc, ExitStack() as ctx:
        io = ctx.enter_context(tc.tile_pool(name="io", bufs=1))
        stat = ctx.enter_context(tc.tile_pool(name="stat", bufs=1))
        work = ctx.enter_context(tc.tile_pool(name="work", bufs=4))
        psb = ctx.enter_context(tc.tile_pool(name="psb", bufs=2, space="PSUM"))

        sb_uT = io.tile([BITS, B], F32)
        nc.sync.dma_start(sb_uT[:], uT[:])
        sb_anch = io.tile([BITS, nr], F32)
        nc.sync.dma_start(sb_anch[:], anch[:])
        sb_posw = io.tile([128, NCH, nr], F32)
        nc.sync.dma_start(sb_posw[:], poswc[:])

        ones_col = stat.tile([128, 1], F32)
        nc.vector.memset(ones_col[:], 1.0)

        # aPm[c][:, q] = ip chunk values minus 0.5 (per-partition bias for U)
        aPm = []
        for c in range(NCH):
            ps_ip = psb.tile([128, nr], F32, tag="ps_ip")
            nc.tensor.matmul(
                ps_ip[:], sb_uT[:, c * 128 : (c + 1) * 128], sb_anch[:],
                start=True, stop=True,
            )
            a = stat.tile([128, nr], F32, tag=f"aPm{c}")
            nc.vector.tensor_scalar_sub(a[:], ps_ip[:], 0.5)
            aPm.append(a)

        # aT[q, :] = full ip row per anchor; staged to DRAM because only DRAM
        # APs support the stride-0 partition-broadcast reads used below
        ps_aT = psb.tile([nr, B], F32, tag="ps_aT")
        nc.tensor.matmul(ps_aT[:], sb_anch[:], sb_uT[:], start=True, stop=True)
        sb_aT = stat.tile([nr, B], F32)
        nc.vector.tensor_copy(sb_aT[:], ps_aT[:])
        aT_dram = nc.dram_tensor("aT_scratch", [nr, B], F32)
        nc.sync.dma_start(aT_dram.ap(), sb_aT[:])

        # per-(anchor, chunk) partial column sums land here
        v_all = stat.tile([128, NCH * nr], F32)

        def row_bcast_ap(ap_row):
            # [1, B] row -> [128, B] partition-broadcast source AP for DMA
            return bass.AP(
                tensor=ap_row.tensor,
                offset=ap_row.offset,
                ap=[[0, 128]] + list(ap_row.ap)[1:],
            )

        for q in range(nr):
            xa = work.tile([128, B], F32, tag="xa")
            nc.sync.dma_start(xa[:], row_bcast_ap(aT_dram.ap()[q : q + 1, :]))
            xn = work.tile([128, B], F32, tag="xn")
            nc.sync.dma_start(xn[:], row_bcast_ap(negwT.ap()[q : q + 1, :]))
            for c in range(NCH):
                u_t = work.tile([128, B], F32, tag="u_t")
                # U = a_j - (a_i - 0.5)
                nc.vector.tensor_scalar(
                    u_t[:], xa[:], aPm[c][:, q : q + 1], None,
                    mybir.AluOpType.subtract,
                )
                cl = work.tile([128, B], F32, tag="cl")
                nc.vector.tensor_scalar(
                    cl[:], u_t[:], 100.0, -50.0,
                    mybir.AluOpType.min, mybir.AluOpType.max,
                )
                # softplus(cl) = max(cl,0) + ln(1 + exp(-|cl|))
                ab = work.tile([128, B], F32, tag="ab")
                nc.scalar.activation(ab[:], cl[:], AF.Abs)
                ex = work.tile([128, B], F32, tag="ex")
                nc.scalar.activation(ex[:], ab[:], AF.Exp, scale=-1.0)
                ln = work.tile([128, B], F32, tag="ln")
                nc.scalar.activation(ln[:], ex[:], AF.Ln, bias=1.0)
                rl = work.tile([128, B], F32, tag="rl")
                nc.vector.tensor_scalar_max(rl[:], cl[:], 0.0)
                sp = work.tile([128, B], F32, tag="sp")
                nc.vector.tensor_add(sp[:], ln[:], rl[:])
                w = work.tile([128, B], F32, tag="w")
                # w = posw_i * SP * neg_j
                nc.vector.scalar_tensor_tensor(
                    w[:], sp[:], sb_posw[:, c, q : q + 1], xn[:],
                    mybir.AluOpType.mult, mybir.AluOpType.mult,
                )
                nc.vector.reduce_sum(
                    v_all[:, q * NCH + c : q * NCH + c + 1], w[:],
                    axis=mybir.AxisListType.X,
                )

        vtot = stat.tile([128, 1], F32)
        nc.vector.reduce_sum(vtot[:], v_all[:], axis=mybir.AxisListType.X)
        ps_out = psb.tile([1, 1], F32, tag="ps_out")
        nc.tensor.matmul(ps_out[:], vtot[:], ones_col[:], start=True, stop=True)
        res = stat.tile([1, 1], F32)
        nc.vector.tensor_copy(res[:], ps_out[:])
        nc.sync.dma_start(out[:], res[:])

    nc.compile()
    return nc


def _loss1_device(u, pos, pair_count, valid, n_valid):
    """Shard valid anchor rows over the cores; run the loss1 program."""
    valid_rows = np.nonzero(valid)[0]
    nr = max(1, (n_valid + N_CORES - 1) // N_CORES)
    uT = np.ascontiguousarray(u.T)  # [BITS, B]

    posw_full = pos.astype(np.float64) / np.where(valid, pair_count, 1.0)[:, None]
    negw_full = 1.0 - pos.astype(np.float64)

    in_maps = []
    for c in range(N_CORES):
        rows = valid_rows[c * nr : (c + 1) * nr]
        anch = np.zeros((BITS, nr), np.float32)
        poswc = np.zeros((128, B // 128, nr), np.float32)
        negwT = np.zeros((nr, B), np.float32)
        for q, r in enumerate(rows):
            anch[:, q] = u[r]
            poswc[:, :, q] = posw_full[r].astype(np.float32).reshape(B // 128, 128).T
            negwT[q, :] = negw_full[r].astype(np.float32)
        in_maps.append(
            {
                "uT": uT,
                "anch": anch,
                "poswc": np.ascontiguousarray(poswc),
                "negwT": negwT,
            }
        )

    nc = _cached(("loss1", nr), build_loss1_program, nr)
    res = run_bass_kernel_spmd(nc, in_maps, core_ids=list(range(N_CORES)))
    partials = [float(r["l1partial"][0, 0]) for r in res.results]
    return np.float32(float(np.sum(partials)) / float(n_valid))


# revision 3
# speedup vs baseline: 1.5311x; 1.0608x over previous
"""Trainium2 Bass kernel for nn_DTSHLoss_48189533061230.

Reference computation (B=384, BITS=128, NCLS=80):
    ip = u @ u.T
    s  = (yf @ yf.T) > 0            # similarity mask from binary labels
    triple[r,i,j] = clip(ip[r,i] - ip[r,j] - 0.5, -100, 50)
    sp = softplus(-triple); w = pos[:,:,None] * neg[:,None,:]
    loss1 = mean over rows with pos&neg pairs;  loss2 = 0.1*mean((u-sign(u))^2)

Structure exploited (ragged_sequence): with NCLS=80 random bits per row,
P(two rows share no class) ~ (3/4)^80 ~ 1e-10, so pair_count == 0 for all
rows => loss1 == 0 exactly. The host does the integer label bookkeeping to
build the ragged schedule; only rows with pairs would get O(B^2) device
work (build_loss1_program, kept for completeness). The always-on device
work is loss2, sharded 8 ways over rows of u (data parallel), with the
scalar partials reduced on the host during unshard.

loss2 device program (per core, shard reshaped to x [128, 48] + canary col):
    DVE  stt1: accum -> t1[p] = sum_c x^2           (fused multiply+row-accum)
    DVE  stt2: accum -> a1[p] = sum_c max(-x, x)    (= sum |x|)
    Pool copy: canary column -> colcat[:, 2]
    Sync DMA out colcat [128, 3]
Host: partial = sum(t1) - 2*sum(a1);  loss2 = 0.1*(sum partials + N - n0)/N.

The out-DMA completion is NOT awaited on-device (awaiting the 16 per-engine
HBM write receipts costs ~5us). Instead a per-attempt random canary vector
rides the full input-DMA -> SBUF -> output-DMA pipeline; the host validates
it bit-exactly and reruns the program on mismatch (each output row's 12
bytes travel in one DMA packet, so a row's canary validates its data).

The Bass-constructor preamble (const memsets, register inits, all-engine
barrier) is pruned from the emitted program: this kernel uses no const APs
or register ops, and the NRT preamble already synchronizes the engines.
"""

import numpy as np

import concourse.bass as bass
import concourse.mybir as mybir
from concourse.bass_utils import run_bass_kernel_spmd

N_CORES = 8
B, BITS, NCLS = 384, 128, 80
F32 = mybir.dt.float32
ALU = mybir.AluOpType
AF = mybir.ActivationFunctionType

P = 128               # SBUF partitions used by the loss2 program
RPC = B // N_CORES    # 48 anchor rows per core
RF = BITS * RPC // P  # 48 free-dim columns per core


def build_loss2_program(rows_per_core: int = RPC, early_out: bool = True):
    R = BITS * rows_per_core // P
    nc = bass.Bass(
        "TRN2", target_bir_lowering=False, debug=False, num_devices=N_CORES
    )

    # Drop the constructor-emitted preamble (const memsets + register inits +
    # all-engine barrier): nothing in this program reads const APs or engine
    # registers, and NRT's own preamble barrier already gates user code.
    insts = nc.main_func.blocks[0].instructions
    insts[:] = [
        ins
        for ins in insts
        if type(ins).__name__
        not in ("InstMemset", "InstDrain", "InstEventSemaphore", "InstRegisterMove")
    ]

    ut = nc.dram_tensor("ut", [P, R + 1], F32, kind="ExternalInput")
    out = nc.dram_tensor("partial", [P, 3], F32, kind="ExternalOutput")

    with (
        nc.sbuf_tensor([P, R + 1], F32) as x,
        nc.sbuf_tensor([P, R], F32) as junk,
        nc.sbuf_tensor([P, R], F32) as junk2,
        nc.sbuf_tensor([P, 3], F32) as colcat,
        nc.semaphore() as dma_sem,
        nc.semaphore() as v_sem,
    ):
        nc.sync.dma_start(x[:], ut[:]).then_inc(dma_sem, 16)

        nc.vector.wait_ge(dma_sem, 16)
        nc.vector.scalar_tensor_tensor(
            junk[:], x[:, 0:R], 1.0, x[:, 0:R], ALU.mult, ALU.mult,
            accum_out=colcat[:, 0:1],
        ).then_inc(v_sem, 1)
        nc.vector.scalar_tensor_tensor(
            junk2[:], x[:, 0:R], -1.0, x[:, 0:R], ALU.mult, ALU.max,
            accum_out=colcat[:, 1:2],
        ).then_inc(v_sem, 1)

        if early_out:
            # canary written LAST on DVE: if the out-DMA's SDMA read beats the
            # compute, every affected row carries a stale canary and the host
            # retries. Sync issues the out-DMA as soon as the input lands; the
            # ~1.3us HWDGE ring latency puts the SDMA colcat read ~700ns after
            # the DVE chain completes.
            nc.vector.wait_ge(v_sem, 2)
            nc.vector.tensor_copy(colcat[:, 2:3], x[:, R : R + 1]).then_inc(
                v_sem, 1
            )
            nc.sync.wait_ge(dma_sem, 16)
            nc.sync.dma_start(out[:], colcat[:]).then_inc(dma_sem, 16)
        else:
            nc.gpsimd.wait_ge(dma_sem, 16)
            nc.gpsimd.tensor_copy(colcat[:, 2:3], x[:, R : R + 1]).then_inc(
                v_sem, 1
            )
            nc.sync.wait_ge(v_sem, 3)
            nc.sync.dma_start(out[:], colcat[:]).then_inc(dma_sem, 16)

    return nc


_program_cache: dict = {}


def _cached(key, builder, *args):
    if key not in _program_cache:
        _program_cache[key] = builder(*args)
    return _program_cache[key]


def _loss2_device(u: np.ndarray) -> float:
    """Run the sharded loss2 program; returns sum((x-sign(x))^2) over all of u."""
    rng = np.random.default_rng()
    shards = [
        u[c * RPC : (c + 1) * RPC, :].reshape(P, RF) for c in range(N_CORES)
    ]

    total = None
    for _attempt in range(8):
        # first attempts use the racy-fast program; if its canary keeps
        # failing, fall back to the conservative one (out-DMA gated on the
        # DVE completion semaphore)
        nc = _cached(
            ("loss2", RPC, _attempt < 3), build_loss2_program, RPC, _attempt < 3
        )
        cans = [
            rng.standard_normal((P, 1)).astype(np.float32) for _ in range(N_CORES)
        ]
        in_maps = [
            {"ut": np.ascontiguousarray(np.concatenate([s, c], axis=1))}
            for s, c in zip(shards, cans)
        ]
        res = run_bass_kernel_spmd(nc, in_maps, core_ids=list(range(N_CORES)))
        ok = True
        tot = 0.0
        for c in range(N_CORES):
            arr = np.asarray(res.results[c]["partial"], dtype=np.float64)
            if not np.array_equal(
                np.asarray(res.results[c]["partial"])[:, 2:3], cans[c]
            ):
                ok = False  # stale/partial output observed; rerun
                break
            tot += arr[:, 0].sum() - 2.0 * arr[:, 1].sum()
        if ok:
            total = tot
            break
    assert total is not None, "loss2 device outputs failed canary validation"

    n_zero = int(np.count_nonzero(u == 0.0))  # x^2-2|x|+1 miscounts these as 1
    return float(total) + float(B * BITS - n_zero)


def kernel(u: np.ndarray, y: np.ndarray) -> np.ndarray:
    u = np.ascontiguousarray(np.asarray(u, dtype=np.float32))
    y = np.asarray(y, dtype=np.int32)
    assert u.shape == (B, BITS) and y.shape == (B, NCLS)

    # ---- host-side ragged schedule bookkeeping (integer label math) ----
    yy = y.astype(np.int64) @ y.astype(np.int64).T
    pos = yy > 0  # [B, B] bool, includes self unless the row is all-zero
    n_pos = pos.sum(1)
    n_neg = B - n_pos
    pair_count = (n_pos * n_neg).astype(np.float64)
    valid = pair_count > 0
    n_valid = int(valid.sum())

    # ---- loss2: always on device, 8-way data parallel over rows ----
    total = _loss2_device(u)
    loss2 = np.float32(0.1 * (total / (B * BITS)))

    loss1 = np.float32(0.0)
    if n_valid > 0:
        loss1 = _loss1_device(u, pos, pair_count, valid, n_valid)

    return np.array(loss1 + loss2, dtype=np.float32)


# ---------------------------------------------------------------------------
# loss1 path: only reachable if some row has both a positive and a negative
# partner (probability ~1e-10 per row pair set under the reference input
# distribution). Kept for correctness on adversarial inputs.
# ---------------------------------------------------------------------------


def build_loss1_program(nr: int):
    """Per-core loss1 partial over `nr` assigned anchor rows (padded).

    For each anchor q with ip-row a (a_i = <u_r, u_i>):
        U[i, j]  = a_j - a_i + 0.5                 == -(triple[r, i, j])
        C        = clip(U, -50, 100)               (mirror of clip(t,-100,50))
        SP       = softplus(C)                     == softplus(-clip(t))
        contrib  = sum_ij posw_i * SP[i, j] * neg_j   (posw = pos/pair_count)
    partial = sum over anchors; the host reduces partials / count.
    """
    from contextlib import ExitStack

    import concourse.bacc as bacc
    import concourse.tile as tile

    nc = bacc.Bacc(
        "TRN2", target_bir_lowering=False, debug=False, num_devices=N_CORES
    )
    uT = nc.dram_tensor("uT", [BITS, B], F32, kind="ExternalInput")
    anch = nc.dram_tensor("anch", [BITS, nr], F32, kind="ExternalInput")
    poswc = nc.dram_tensor("poswc", [128, 3, nr], F32, kind="ExternalInput")
    negwT = nc.dram_tensor("negwT", [nr, B], F32, kind="ExternalInput")
    out = nc.dram_tensor("l1partial", [1, 1], F32, kind="ExternalOutput")
    NCH = B // 128  # 3 partition chunks of the i axis

    with tile.TileContext(nc) as tc, ExitStack() as ctx:
        io = ctx.enter_context(tc.tile_pool(name="io", bufs=1))
        stat = ctx.enter_context(tc.tile_pool(name="stat", bufs=1))
        work = ctx.enter_context(tc.tile_pool(name="work", bufs=4))
        psb = ctx.enter_context(tc.tile_pool(name="psb", bufs=2, space="PSUM"))

        sb_uT = io.tile([BITS, B], F32)
        nc.sync.dma_start(sb_uT[:], uT[:])
        sb_anch = io.tile([BITS, nr], F32)
        nc.sync.dma_start(sb_anch[:], anch[:])
        sb_posw = io.tile([128, NCH, nr], F32)
        nc.sync.dma_start(sb_posw[:], poswc[:])

        ones_col = stat.tile([128, 1], F32)
        nc.vector.memset(ones_col[:], 1.0)

        # aPm[c][:, q] = ip chunk values minus 0.5 (per-partition bias for U)
        aPm = []
        for c in range(NCH):
            ps_ip = psb.tile([128, nr], F32, tag="ps_ip")
            nc.tensor.matmul(
                ps_ip[:], sb_uT[:, c * 128 : (c + 1) * 128], sb_anch[:],
                start=True, stop=True,
            )
            a = stat.tile([128, nr], F32, tag=f"aPm{c}")
            nc.vector.tensor_scalar_sub(a[:], ps_ip[:], 0.5)
            aPm.append(a)

        # aT[q, :] = full ip row per anchor; staged to DRAM because only DRAM
        # APs support the stride-0 partition-broadcast reads used below
        ps_aT = psb.tile([nr, B], F32, tag="ps_aT")
        nc.tensor.matmul(ps_aT[:], sb_anch[:], sb_uT[:], start=True, stop=True)
        sb_aT = stat.tile([nr, B], F32)
        nc.vector.tensor_copy(sb_aT[:], ps_aT[:])
        aT_dram = nc.dram_tensor("aT_scratch", [nr, B], F32)
        nc.sync.dma_start(aT_dram.ap(), sb_aT[:])

        # per-(anchor, chunk) partial column sums land here
        v_all = stat.tile([128, NCH * nr], F32)

        def row_bcast_ap(ap_row):
            # [1, B] row -> [128, B] partition-broadcast source AP for DMA
            return bass.AP(
                tensor=ap_row.tensor,
                offset=ap_row.offset,
                ap=[[0, 128]] + list(ap_row.ap)[1:],
            )

        for q in range(nr):
            xa = work.tile([128, B], F32, tag="xa")
            nc.sync.dma_start(xa[:], row_bcast_ap(aT_dram.ap()[q : q + 1, :]))
            xn = work.tile([128, B], F32, tag="xn")
            nc.sync.dma_start(xn[:], row_bcast_ap(negwT.ap()[q : q + 1, :]))
            for c in range(NCH):
                u_t = work.tile([128, B], F32, tag="u_t")
                # U = a_j - (a_i - 0.5)
                nc.vector.tensor_scalar(
                    u_t[:], xa[:], aPm[c][:, q : q + 1], None,
                    mybir.AluOpType.subtract,
                )
                cl = work.tile([128, B], F32, tag="cl")
                nc.vector.tensor_scalar(
                    cl[:], u_t[:], 100.0, -50.0,
                    mybir.AluOpType.min, mybir.AluOpType.max,
                )
                # softplus(cl) = max(cl,0) + ln(1 + exp(-|cl|))
                ab = work.tile([128, B], F32, tag="ab")
                nc.scalar.activation(ab[:], cl[:], AF.Abs)
                ex = work.tile([128, B], F32, tag="ex")
                nc.scalar.activation(ex[:], ab[:], AF.Exp, scale=-1.0)
                ln = work.tile([128, B], F32, tag="ln")
                nc.scalar.activation(ln[:], ex[:], AF.Ln, bias=1.0)
                rl = work.tile([128, B], F32, tag="rl")
                nc.vector.tensor_scalar_max(rl[:], cl[:], 0.0)
                sp = work.tile([128, B], F32, tag="sp")
                nc.vector.tensor_add(sp[:], ln[:], rl[:])
                w = work.tile([128, B], F32, tag="w")
                # w = posw_i * SP * neg_j
                nc.vector.scalar_tensor_tensor(
                    w[:], sp[:], sb_posw[:, c, q : q + 1], xn[:],
                    mybir.AluOpType.mult, mybir.AluOpType.mult,
                )
                nc.vector.reduce_sum(
                    v_all[:, q * NCH + c : q * NCH + c + 1], w[:],
                    axis=mybir.AxisListType.X,
                )

        vtot = stat.tile([128, 1], F32)
        nc.vector.reduce_sum(vtot[:], v_all[:], axis=mybir.AxisListType.X)
        ps_out = psb.tile([1, 1], F32, tag="ps_out")
        nc.tensor.matmul(ps_out[:], vtot[:], ones_col[:], start=True, stop=True)
        res = stat.tile([1, 1], F32)
        nc.vector.tensor_copy(res[:], ps_out[:])
        nc.sync.dma_start(out[:], res[:])

    nc.compile()
    return nc


def _loss1_device(u, pos, pair_count, valid, n_valid):
    """Shard valid anchor rows over the cores; run the loss1 program."""
    valid_rows = np.nonzero(valid)[0]
    nr = max(1, (n_valid + N_CORES - 1) // N_CORES)
    uT = np.ascontiguousarray(u.T)  # [BITS, B]

    posw_full = pos.astype(np.float64) / np.where(valid, pair_count, 1.0)[:, None]
    negw_full = 1.0 - pos.astype(np.float64)

    in_maps = []
    for c in range(N_CORES):
        rows = valid_rows[c * nr : (c + 1) * nr]
        anch = np.zeros((BITS, nr), np.float32)
        poswc = np.zeros((128, B // 128, nr), np.float32)
        negwT = np.zeros((nr, B), np.float32)
        for q, r in enumerate(rows):
            anch[:, q] = u[r]
            poswc[:, :, q] = posw_full[r].astype(np.float32).reshape(B // 128, 128).T
            negwT[q, :] = negw_full[r].astype(np.float32)
        in_maps.append(
            {
                "uT": uT,
                "anch": anch,
                "poswc": np.ascontiguousarray(poswc),
                "negwT": negwT,
            }
        )

    nc = _cached(("loss1", nr), build_loss1_program, nr)
    res = run_bass_kernel_spmd(nc, in_maps, core_ids=list(range(N_CORES)))
    partials = [float(r["l1partial"][0, 0]) for r in res.results]
    return np.float32(float(np.sum(partials)) / float(n_valid))


# revision 4
# speedup vs baseline: 1.5456x; 1.0095x over previous
"""Trainium2 Bass kernel for nn_DTSHLoss_48189533061230.

Reference computation (B=384, BITS=128, NCLS=80):
    ip = u @ u.T
    s  = (yf @ yf.T) > 0            # similarity mask from binary labels
    triple[r,i,j] = clip(ip[r,i] - ip[r,j] - 0.5, -100, 50)
    sp = softplus(-triple); w = pos[:,:,None] * neg[:,None,:]
    loss1 = mean over rows with pos&neg pairs;  loss2 = 0.1*mean((u-sign(u))^2)

Structure exploited (ragged_sequence): with NCLS=80 random bits per row,
P(two rows share no class) ~ (3/4)^80 ~ 1e-10, so pair_count == 0 for all
rows => loss1 == 0 exactly. The host does the integer label bookkeeping to
build the ragged schedule; only rows with pairs would get O(B^2) device
work (build_loss1_program, kept for completeness). The always-on device
work is loss2, sharded 8 ways over rows of u (data parallel), with the
scalar partials reduced on the host during unshard.

loss2 device program (per core, shard reshaped to x [128, 48] + canary col):
    DVE  stt1: accum -> t1[p] = sum_c x^2           (fused multiply+row-accum)
    DVE  stt2: accum -> a1[p] = sum_c max(-x, x)    (= sum |x|)
    Pool copy: canary column -> colcat[:, 2]
    Sync DMA out colcat [128, 3]
Host: partial = sum(t1) - 2*sum(a1);  loss2 = 0.1*(sum partials + N - n0)/N.

The out-DMA completion is NOT awaited on-device (awaiting the 16 per-engine
HBM write receipts costs ~5us). Instead a per-attempt random canary vector
rides the full input-DMA -> SBUF -> output-DMA pipeline; the host validates
it bit-exactly and reruns the program on mismatch (each output row's 12
bytes travel in one DMA packet, so a row's canary validates its data).

The Bass-constructor preamble (const memsets, register inits, all-engine
barrier) is pruned from the emitted program: this kernel uses no const APs
or register ops, and the NRT preamble already synchronizes the engines.
"""

import numpy as np

import concourse.bass as bass
import concourse.mybir as mybir
from concourse.bass_utils import run_bass_kernel_spmd

N_CORES = 8
B, BITS, NCLS = 384, 128, 80
F32 = mybir.dt.float32
ALU = mybir.AluOpType
AF = mybir.ActivationFunctionType

P = 128               # SBUF partitions used by the loss2 program
RPC = B // N_CORES    # 48 anchor rows per core
RF = BITS * RPC // P  # 48 free-dim columns per core


def build_loss2_program(rows_per_core: int = RPC, early_out: bool = True):
    R = BITS * rows_per_core // P
    nc = bass.Bass(
        "TRN2", target_bir_lowering=False, debug=False, num_devices=N_CORES
    )

    # Drop the constructor-emitted preamble (const memsets + register inits +
    # all-engine barrier): nothing in this program reads const APs or engine
    # registers, and NRT's own preamble barrier already gates user code.
    insts = nc.main_func.blocks[0].instructions
    insts[:] = [
        ins
        for ins in insts
        if type(ins).__name__
        not in ("InstMemset", "InstDrain", "InstEventSemaphore", "InstRegisterMove")
    ]

    ut = nc.dram_tensor("ut", [P, R + 1], F32, kind="ExternalInput")
    out = nc.dram_tensor("partial", [P, 3], F32, kind="ExternalOutput")

    with (
        nc.sbuf_tensor([P, R + 1], F32) as x,
        nc.sbuf_tensor([P, R], F32) as junk,
        nc.sbuf_tensor([P, R], F32) as junk2,
        nc.sbuf_tensor([P, 3], F32) as colcat,
        nc.semaphore() as dma_sem,
        nc.semaphore() as v_sem,
    ):
        nc.sync.dma_start(x[:], ut[:]).then_inc(dma_sem, 16)

        nc.vector.wait_ge(dma_sem, 16)
        nc.vector.scalar_tensor_tensor(
            junk[:], x[:, 0:R], 1.0, x[:, 0:R], ALU.mult, ALU.mult,
            accum_out=colcat[:, 0:1],
        ).then_inc(v_sem, 1)
        nc.vector.scalar_tensor_tensor(
            junk2[:], x[:, 0:R], -1.0, x[:, 0:R], ALU.mult, ALU.max,
            accum_out=colcat[:, 1:2],
        ).then_inc(v_sem, 1)

        if early_out:
            # canary written LAST on DVE: if the out-DMA's SDMA read beats the
            # compute, every affected row carries a stale canary and the host
            # retries. Sync issues the out-DMA as soon as the input lands; the
            # ~1.3us HWDGE ring latency puts the SDMA colcat read ~700ns after
            # the DVE chain completes.
            nc.vector.wait_ge(v_sem, 2)
            nc.vector.tensor_copy(colcat[:, 2:3], x[:, R : R + 1]).then_inc(
                v_sem, 1
            )
            nc.sync.wait_ge(dma_sem, 8)
            nc.sync.dma_start(out[:], colcat[:]).then_inc(dma_sem, 16)
        else:
            nc.gpsimd.wait_ge(dma_sem, 16)
            nc.gpsimd.tensor_copy(colcat[:, 2:3], x[:, R : R + 1]).then_inc(
                v_sem, 1
            )
            nc.sync.wait_ge(v_sem, 3)
            nc.sync.dma_start(out[:], colcat[:]).then_inc(dma_sem, 16)

    return nc


_program_cache: dict = {}


def _cached(key, builder, *args):
    if key not in _program_cache:
        _program_cache[key] = builder(*args)
    return _program_cache[key]


def _loss2_device(u: np.ndarray) -> float:
    """Run the sharded loss2 program; returns sum((x-sign(x))^2) over all of u."""
    rng = np.random.default_rng()
    shards = [
        u[c * RPC : (c + 1) * RPC, :].reshape(P, RF) for c in range(N_CORES)
    ]

    total = None
    for _attempt in range(8):
        # first attempts use the racy-fast program; if its canary keeps
        # failing, fall back to the conservative one (out-DMA gated on the
        # DVE completion semaphore)
        nc = _cached(
            ("loss2", RPC, _attempt < 3), build_loss2_program, RPC, _attempt < 3
        )
        cans = [
            rng.standard_normal((P, 1)).astype(np.float32) for _ in range(N_CORES)
        ]
        in_maps = [
            {"ut": np.ascontiguousarray(np.concatenate([s, c], axis=1))}
            for s, c in zip(shards, cans)
        ]
        res = run_bass_kernel_spmd(nc, in_maps, core_ids=list(range(N_CORES)))
        ok = True
        tot = 0.0
        for c in range(N_CORES):
            arr = np.asarray(res.results[c]["partial"], dtype=np.float64)
            if not np.array_equal(
                np.asarray(res.results[c]["partial"])[:, 2:3], cans[c]
            ):
                ok = False  # stale/partial output observed; rerun
                break
            tot += arr[:, 0].sum() - 2.0 * arr[:, 1].sum()
        if ok:
            total = tot
            break
    assert total is not None, "loss2 device outputs failed canary validation"

    n_zero = int(np.count_nonzero(u == 0.0))  # x^2-2|x|+1 miscounts these as 1
    return float(total) + float(B * BITS - n_zero)


def kernel(u: np.ndarray, y: np.ndarray) -> np.ndarray:
    u = np.ascontiguousarray(np.asarray(u, dtype=np.float32))
    y = np.asarray(y, dtype=np.int32)
    assert u.shape == (B, BITS) and y.shape == (B, NCLS)

    # ---- host-side ragged schedule bookkeeping (integer label math) ----
    yy = y.astype(np.int64) @ y.astype(np.int64).T
    pos = yy > 0  # [B, B] bool, includes self unless the row is all-zero
    n_pos = pos.sum(1)
    n_neg = B - n_pos
    pair_count = (n_pos * n_neg).astype(np.float64)
    valid = pair_count > 0
    n_valid = int(valid.sum())

    # ---- loss2: always on device, 8-way data parallel over rows ----
    total = _loss2_device(u)
    loss2 = np.float32(0.1 * (total / (B * BITS)))

    loss1 = np.float32(0.0)
    if n_valid > 0:
        loss1 = _loss1_device(u, pos, pair_count, valid, n_valid)

    return np.array(loss1 + loss2, dtype=np.float32)


# ---------------------------------------------------------------------------
# loss1 path: only reachable if some row has both a positive and a negative
# partner (probability ~1e-10 per row pair set under the reference input
# distribution). Kept for correctness on adversarial inputs.
# ---------------------------------------------------------------------------


def build_loss1_program(nr: int):
    """Per-core loss1 partial over `nr` assigned anchor rows (padded).

    For each anchor q with ip-row a (a_i = <u_r, u_i>):
        U[i, j]  = a_j - a_i + 0.5                 == -(triple[r, i, j])
        C        = clip(U, -50, 100)               (mirror of clip(t,-100,50))
        SP       = softplus(C)                     == softplus(-clip(t))
        contrib  = sum_ij posw_i * SP[i, j] * neg_j   (posw = pos/pair_count)
    partial = sum over anchors; the host reduces partials / count.
    """
    from contextlib import ExitStack

    import concourse.bacc as bacc
    import concourse.tile as tile

    nc = bacc.Bacc(
        "TRN2", target_bir_lowering=False, debug=False, num_devices=N_CORES
    )
    uT = nc.dram_tensor("uT", [BITS, B], F32, kind="ExternalInput")
    anch = nc.dram_tensor("anch", [BITS, nr], F32, kind="ExternalInput")
    poswc = nc.dram_tensor("poswc", [128, 3, nr], F32, kind="ExternalInput")
    negwT = nc.dram_tensor("negwT", [nr, B], F32, kind="ExternalInput")
    out = nc.dram_tensor("l1partial", [1, 1], F32, kind="ExternalOutput")
    NCH = B // 128  # 3 partition chunks of the i axis

    with tile.TileContext(nc) as tc, ExitStack() as ctx:
        io = ctx.enter_context(tc.tile_pool(name="io", bufs=1))
        stat = ctx.enter_context(tc.tile_pool(name="stat", bufs=1))
        work = ctx.enter_context(tc.tile_pool(name="work", bufs=4))
        psb = ctx.enter_context(tc.tile_pool(name="psb", bufs=2, space="PSUM"))

        sb_uT = io.tile([BITS, B], F32)
        nc.sync.dma_start(sb_uT[:], uT[:])
        sb_anch = io.tile([BITS, nr], F32)
        nc.sync.dma_start(sb_anch[:], anch[:])
        sb_posw = io.tile([128, NCH, nr], F32)
        nc.sync.dma_start(sb_posw[:], poswc[:])

        ones_col = stat.tile([128, 1], F32)
        nc.vector.memset(ones_col[:], 1.0)

        # aPm[c][:, q] = ip chunk values minus 0.5 (per-partition bias for U)
        aPm = []
        for c in range(NCH):
            ps_ip = psb.tile([128, nr], F32, tag="ps_ip")
            nc.tensor.matmul(
                ps_ip[:], sb_uT[:, c * 128 : (c + 1) * 128], sb_anch[:],
                start=True, stop=True,
            )
            a = stat.tile([128, nr], F32, tag=f"aPm{c}")
            nc.vector.tensor_scalar_sub(a[:], ps_ip[:], 0.5)
            aPm.append(a)

        # aT[q, :] = full ip row per anchor; staged to DRAM because only DRAM
        # APs support the stride-0 partition-broadcast reads used below
        ps_aT = psb.tile([nr, B], F32, tag="ps_aT")
        nc.tensor.matmul(ps_aT[:], sb_anch[:], sb_uT[:], start=True, stop=True)
        sb_aT = stat.tile([nr, B], F32)
        nc.vector.tensor_copy(sb_aT[:], ps_aT[:])
        aT_dram = nc.dram_tensor("aT_scratch", [nr, B], F32)
        nc.sync.dma_start(aT_dram.ap(), sb_aT[:])

        # per-(anchor, chunk) partial column sums land here
        v_all = stat.tile([128, NCH * nr], F32)

        def row_bcast_ap(ap_row):
            # [1, B] row -> [128, B] partition-broadcast source AP for DMA
            return bass.AP(
                tensor=ap_row.tensor,
                offset=ap_row.offset,
                ap=[[0, 128]] + list(ap_row.ap)[1:],
            )

        for q in range(nr):
            xa = work.tile([128, B], F32, tag="xa")
            nc.sync.dma_start(xa[:], row_bcast_ap(aT_dram.ap()[q : q + 1, :]))
            xn = work.tile([128, B], F32, tag="xn")
            nc.sync.dma_start(xn[:], row_bcast_ap(negwT.ap()[q : q + 1, :]))
            for c in range(NCH):
                u_t = work.tile([128, B], F32, tag="u_t")
                # U = a_j - (a_i - 0.5)
                nc.vector.tensor_scalar(
                    u_t[:], xa[:], aPm[c][:, q : q + 1], None,
                    mybir.AluOpType.subtract,
                )
                cl = work.tile([128, B], F32, tag="cl")
                nc.vector.tensor_scalar(
                    cl[:], u_t[:], 100.0, -50.0,
                    mybir.AluOpType.min, mybir.AluOpType.max,
                )
                # softplus(cl) = max(cl,0) + ln(1 + exp(-|cl|))
                ab = work.tile([128, B], F32, tag="ab")
                nc.scalar.activation(ab[:], cl[:], AF.Abs)
                ex = work.tile([128, B], F32, tag="ex")
                nc.scalar.activation(ex[:], ab[:], AF.Exp, scale=-1.0)
                ln = work.tile([128, B], F32, tag="ln")
                nc.scalar.activation(ln[:], ex[:], AF.Ln, bias=1.0)
                rl = work.tile([128, B], F32, tag="rl")
                nc.vector.tensor_scalar_max(rl[:], cl[:], 0.0)
                sp = work.tile([128, B], F32, tag="sp")
                nc.vector.tensor_add(sp[:], ln[:], rl[:])
                w = work.tile([128, B], F32, tag="w")
                # w = posw_i * SP * neg_j
                nc.vector.scalar_tensor_tensor(
                    w[:], sp[:], sb_posw[:, c, q : q + 1], xn[:],
                    mybir.AluOpType.mult, mybir.AluOpType.mult,
                )
                nc.vector.reduce_sum(
                    v_all[:, q * NCH + c : q * NCH + c + 1], w[:],
                    axis=mybir.AxisListType.X,
                )

        vtot = stat.tile([128, 1], F32)
        nc.vector.reduce_sum(vtot[:], v_all[:], axis=mybir.AxisListType.X)
        ps_out = psb.tile([1, 1], F32, tag="ps_out")
        nc.tensor.matmul(ps_out[:], vtot[:], ones_col[:], start=True, stop=True)
        res = stat.tile([1, 1], F32)
        nc.vector.tensor_copy(res[:], ps_out[:])
        nc.sync.dma_start(out[:], res[:])

    nc.compile()
    return nc


def _loss1_device(u, pos, pair_count, valid, n_valid):
    """Shard valid anchor rows over the cores; run the loss1 program."""
    valid_rows = np.nonzero(valid)[0]
    nr = max(1, (n_valid + N_CORES - 1) // N_CORES)
    uT = np.ascontiguousarray(u.T)  # [BITS, B]

    posw_full = pos.astype(np.float64) / np.where(valid, pair_count, 1.0)[:, None]
    negw_full = 1.0 - pos.astype(np.float64)

    in_maps = []
    for c in range(N_CORES):
        rows = valid_rows[c * nr : (c + 1) * nr]
        anch = np.zeros((BITS, nr), np.float32)
        poswc = np.zeros((128, B // 128, nr), np.float32)
        negwT = np.zeros((nr, B), np.float32)
        for q, r in enumerate(rows):
            anch[:, q] = u[r]
            poswc[:, :, q] = posw_full[r].astype(np.float32).reshape(B // 128, 128).T
            negwT[q, :] = negw_full[r].astype(np.float32)
        in_maps.append(
            {
                "uT": uT,
                "anch": anch,
                "poswc": np.ascontiguousarray(poswc),
                "negwT": negwT,
            }
        )

    nc = _cached(("loss1", nr), build_loss1_program, nr)
    res = run_bass_kernel_spmd(nc, in_maps, core_ids=list(range(N_CORES)))
    partials = [float(r["l1partial"][0, 0]) for r in res.results]
    return np.float32(float(np.sum(partials)) / float(n_valid))
